# revision 1
# baseline (speedup 1.0000x reference)
"""Trainium2 Bass kernel for nn_MultiHeadAttention_84791244358011.

Linear (ELU feature-map) attention:
    x_norm = LayerNorm(x)                      # eps=1e-12
    q = x_norm @ Wq.T + bq ; k,v = x @ W.T + b # per-head [S, 64]
    eq/ek = l2norm(elu(q/k)) per token over head_dim
    kv = ek^T @ v per head [64, 64]; ctx = eq @ kv / 8
    out = ctx @ Wo.T + bo + x

Sharding: data-parallel over batch B=8 — one batch element per NeuronCore,
no collectives. Weights are pre-transposed host-side (static prep):
    wqt[i,j] = Wq[j,i]*gamma[i]; wkt/wvt = W.T; wot = Wo.T*(1/sqrt(64))
    bq_eff = bq + beta @ Wq.T
so every on-device matmul contracts over the SBUF partition dim. Matmuls
run in float32r (full PE rate, ~2^-13 operand rounding).

Per core, two passes over the 32 token-tiles (128 tokens each):
  pass A: load x tile -> LN stats -> z=(x-mu)*rstd; PE-transpose x and z;
          k/v projections from x^T (biases folded in as K=1 rank-1
          matmuls); elu+l2norm on k; accumulate per-head kv state in
          PSUM; spill z^T to DRAM scratch.
  pass B: reload z^T -> q projection -> elu+l2norm -> PE-transpose eq;
          per 512-token chunk ctx^T = kv @ eq^T (row-tiled 64x64 pairs);
          out = ctx^T.T @ wot (+bo via rank-1) + x.

Scheduling notes (from CoreSim engine-occupancy analysis):
  - all rsqrt/sqrt via Newton on DVE (quake seed + 2 iters) so the scalar
    engine stays on one ACT table set (exp_and_others) — table loads are
    1.28us each;
  - k/v/q/o PSUM tiles are bank-sized halves with bufs=2 so the PE runs
    ahead while the DVE drains the previous tile's PSUM;
  - DMA load is spread across the SP/ACT HWDGE queues and the gpsimd
    SWDGE queue;
  - PSUM start=True pends-zero the whole 2KB bank: only the first matmul
    per bank may set it.
"""

import numpy as np

import concourse.bass as bass
import concourse.mybir as mybir
import concourse.tile as tile
from concourse import bacc
from concourse.masks import make_identity

B, S, HID = 8, 4096, 1024
NH, HD = 16, 64
P = 128
NT = S // P            # 32 token tiles
NC = HID // P          # 8 feature chunks
HNH = NH // 2          # heads per psum half
CHUNK = 4              # token tiles per ctx chunk (512 tokens)
NCHUNKS = NT // CHUNK
LN_EPS = 1e-12

F32 = mybir.dt.float32
F32R = mybir.dt.float32r
I32 = mybir.dt.int32
AF = mybir.ActivationFunctionType
OP = mybir.AluOpType


def _rsqrt(nc, pool, consts, src, n, name):
    """1/sqrt(src[:, :n]) on DVE: quake-III seed + 2 Newton iterations."""
    magic_b, one_b = consts
    e = nc.vector
    shi = pool.tile([P, n], I32, tag=f"rq_sh{n}", bufs=4, name=f"{name}_shi")
    e.tensor_tensor(shi, src.bitcast(I32), one_b[:, 0:1].to_broadcast((P, n)),
                    OP.arith_shift_right)
    y0i = pool.tile([P, n], I32, tag=f"rq_y0{n}", bufs=4, name=f"{name}_y0i")
    e.tensor_tensor(y0i, magic_b[:, 0:1].to_broadcast((P, n)), shi, OP.subtract)
    h = pool.tile([P, n], F32, tag=f"rq_h{n}", bufs=4, name=f"{name}_h")
    e.tensor_scalar(h, src, -0.5, None, OP.mult)
    cur = y0i[:].bitcast(F32)
    for it in range(2):
        t = pool.tile([P, n], F32, tag=f"rq_t{n}_{it}", bufs=4,
                      name=f"{name}_t{it}")
        e.tensor_tensor(t, cur, cur, OP.mult)
        e.tensor_tensor(t, t, h, OP.mult)
        e.tensor_scalar(t, t, 1.5, None, OP.add)
        y = pool.tile([P, n], F32, tag=f"rq_y{n}_{it}", bufs=4,
                      name=f"{name}_y{it}")
        e.tensor_tensor(y, cur, t, OP.mult)
        cur = y
    return cur


def _elu_half(nc, pool, ps_half, bias_rep, half, raw, name):
    """raw[:, half-heads] = elu(ps_half + bias) = (max(x,0)-1) + exp(min(x,0))."""
    sl = slice(half * 512, (half + 1) * 512)
    xb = pool.tile([P, HID // 2], F32, tag="eb", bufs=3, name=f"{name}_xb")
    nc.vector.tensor_tensor(xb, ps_half, bias_rep[:, sl], OP.add)
    r = pool.tile([P, HID // 2], F32, tag="er", bufs=3, name=f"{name}_r")
    nc.scalar.activation(r, xb, AF.Relu, scale=-1.0)
    e = pool.tile([P, HID // 2], F32, tag="ee", bufs=3, name=f"{name}_e")
    nc.scalar.activation(e, r, AF.Exp, scale=-1.0)
    m = pool.tile([P, HID // 2], F32, tag="em", bufs=3, name=f"{name}_m")
    nc.vector.tensor_scalar(m, xb, 0.0, 1.0, OP.max, OP.subtract)
    nc.gpsimd.tensor_tensor(
        raw[:, half * HNH:(half + 1) * HNH, :].rearrange("p h d -> p (h d)"),
        m, e, OP.add)


def build_nc(debug=False, loop_n=1):
    nc = bacc.Bacc("TRN2", target_bir_lowering=False, enable_partition_id=False)
    dbg = {}
    if debug:
        dbg["ek0"] = nc.dram_tensor("dbg_ek0", [P, HID], F32, kind="ExternalOutput")
        dbg["v0"] = nc.dram_tensor("dbg_v0", [P, HID], F32, kind="ExternalOutput")
        dbg["kv"] = nc.dram_tensor("dbg_kv", [HD, NH * HD], F32,
                                   kind="ExternalOutput")
        dbg["eq0"] = nc.dram_tensor("dbg_eq0", [P, HID], F32, kind="ExternalOutput")

    x_d = nc.dram_tensor("x", [S, HID], F32, kind="ExternalInput")
    wqt_d = nc.dram_tensor("wqt", [HID, HID], F32, kind="ExternalInput")
    wkt_d = nc.dram_tensor("wkt", [HID, HID], F32, kind="ExternalInput")
    wvt_d = nc.dram_tensor("wvt", [HID, HID], F32, kind="ExternalInput")
    wot_d = nc.dram_tensor("wot", [HID, HID], F32, kind="ExternalInput")
    bq_d = nc.dram_tensor("bq", [1, HID], F32, kind="ExternalInput")
    bk_d = nc.dram_tensor("bk", [1, HID], F32, kind="ExternalInput")
    bv_d = nc.dram_tensor("bv", [1, HID], F32, kind="ExternalInput")
    bo_d = nc.dram_tensor("bo", [1, HID], F32, kind="ExternalInput")
    out_d = nc.dram_tensor("out", [S, HID], F32, kind="ExternalOutput")
    zt_d = nc.dram_tensor("zt_spill", [NT, P, HID], F32)

    import contextlib

    with tile.TileContext(nc) as tc, contextlib.ExitStack() as ctx:
        persist = ctx.enter_context(tc.tile_pool(name="persist", bufs=1))
        wpool = ctx.enter_context(tc.tile_pool(name="weights", bufs=1))

        ident = persist.tile([P, P], F32)
        make_identity(nc, ident)
        ident_r = persist.tile([P, P], F32R)
        nc.scalar.activation(ident_r, ident, AF.Copy)
        magic_t = persist.tile([P, 1], I32)
        nc.gpsimd.memset(magic_t, 0x5F3759DF)
        one_t = persist.tile([P, 1], I32)
        nc.gpsimd.memset(one_t, 1)
        consts = (magic_t, one_t)
        kv_sb = persist.tile([P, HNH * HD], F32R)   # packed kv state
        brep = {}
        for nm, d in (("bq", bq_d), ("bk", bk_d), ("bv", bv_d), ("bo", bo_d)):
            t_ = persist.tile([P, HID], F32, name=f"{nm}_rep")
            h = d.ap()
            nc.gpsimd.dma_start(
                t_, bass.AP(tensor=h.tensor, offset=h.offset,
                            ap=[[0, P], [1, HID]]))
            brep[nm] = t_

        def proj_half(ps, lhs_fn, w_sb, half):
            """ps[P,512] = sum_c lhs(c).T @ w[c, half]."""
            sl = slice(half * 512, (half + 1) * 512)
            for c in range(NC):
                nc.tensor.matmul(ps, lhs_fn(c), w_sb[:, c, sl],
                                 start=(c == 0), stop=(c == NC - 1))

        _loop = tc.For_i(0, loop_n, 1) if loop_n > 1 else contextlib.nullcontext(0)
        with _loop:
            # ------------- pass A: k/v projections + kv state -------------
            wk_sb = wpool.tile([P, NC, HID], F32R, tag="wA", name="wk_sb")
            nc.sync.dma_start(
                wk_sb, wkt_d.ap().rearrange("(c p) j -> p c j", p=P).bitcast(F32R))
            wv_sb = wpool.tile([P, NC, HID], F32R, tag="wB", name="wv_sb")
            nc.sync.dma_start(
                wv_sb, wvt_d.ap().rearrange("(c p) j -> p c j", p=P).bitcast(F32R))

            with tc.tile_pool(name="sbufA", bufs=1) as sa, \
                 tc.tile_pool(name="psumA", bufs=1, space="PSUM") as pa:
                # kv state: [64, NH*64] — every head at partition base 0
                kv_ps = pa.tile([HD, NH * HD], F32, tag="kv", name="kv_ps")

                for t in range(NT):
                    x_t = sa.tile([P, HID], F32, tag="x", bufs=4, name=f"x_{t}")
                    nc.scalar.dma_start(x_t, x_d.ap()[t * P:(t + 1) * P, :])

                    # LayerNorm stats
                    stats = sa.tile([P, 2, 6], F32, tag="st", bufs=4,
                                    name=f"st_{t}")
                    xg = x_t[:].rearrange("p (g d) -> p g d", g=2)
                    for g in range(2):
                        nc.vector.bn_stats(stats[:, g, :], xg[:, g, :])
                    mv = sa.tile([P, 2], F32, tag="mv", bufs=4, name=f"mv_{t}")
                    nc.vector.bn_aggr(mv, stats)
                    vpe = sa.tile([P, 1], F32, tag="sd", bufs=4, name=f"sd_{t}")
                    nc.vector.tensor_scalar(vpe, mv[:, 1:2], LN_EPS, None, OP.add)
                    rstd = _rsqrt(nc, sa, consts, vpe[:], 1, f"rs_{t}")
                    z_t = sa.tile([P, HID], F32, tag="z", bufs=3, name=f"z_{t}")
                    nc.vector.tensor_scalar(
                        z_t, x_t, mv[:, 0:1], rstd, OP.subtract, OP.mult)

                    # transpose x and z (PE), copy out, spill z^T
                    xT = sa.tile([P, NC, P], F32R, tag="xT", bufs=3,
                                 name=f"xT_{t}")
                    zT = sa.tile([P, NC, P], F32, tag="zT", bufs=2,
                                 name=f"zT_{t}")
                    for src, dst in ((x_t, xT), (z_t, zT)):
                        for half in range(2):
                            tp = pa.tile([P, 4 * P], F32, tag="tp", bufs=2,
                                         name=f"tp_{t}_{half}")
                            for b_ in range(4):
                                blk = half * 4 + b_
                                nc.tensor.transpose(
                                    tp[:, b_ * P:(b_ + 1) * P],
                                    src[:, blk * P:(blk + 1) * P], ident)
                            nc.vector.tensor_copy(
                                dst[:].rearrange("p c s -> p (c s)")[
                                    :, half * 4 * P:(half + 1) * 4 * P], tp)
                    nc.sync.dma_start(zt_d.ap()[t],
                                      zT[:].rearrange("p c s -> p (c s)"))

                    def xT_lhs(c, xT=xT):
                        return xT[:, c, :]

                    # k/v projections into half-bank psums; elu on k halves
                    raw = sa.tile([P, NH, HD], F32, tag="eraw", bufs=2,
                                  name=f"ekraw{t}")
                    v_sb = sa.tile([P, NH, HD], F32R, tag="vsb", bufs=2,
                                   name=f"v_sb{t}")
                    for half in range(2):
                        k_ps = pa.tile([P, 512], F32, tag="kh", bufs=2,
                                       name=f"k_ps{t}_{half}")
                        proj_half(k_ps, xT_lhs, wk_sb, half)
                        _elu_half(nc, sa, k_ps, brep["bk"], half, raw,
                                  f"ek{t}_{half}")
                        v_ps = pa.tile([P, 512], F32, tag="vh", bufs=2,
                                       name=f"v_ps{t}_{half}")
                        proj_half(v_ps, xT_lhs, wv_sb, half)
                        nc.vector.tensor_tensor(
                            v_sb[:, half * HNH:(half + 1) * HNH, :].rearrange(
                                "p h d -> p (h d)"), v_ps,
                            brep["bv"][:, half * 512:(half + 1) * 512], OP.add)

                    # per-head l2 norm of elu(k): sumsq -> rsqrt -> scale
                    sq = sa.tile([P, NH, HD], F32, tag="esq", bufs=2,
                                 name=f"sq{t}")
                    nc.scalar.activation(
                        sq[:].rearrange("p h d -> p (h d)"),
                        raw[:].rearrange("p h d -> p (h d)"), AF.Square)
                    ss = sa.tile([P, NH], F32, tag="ess", bufs=4, name=f"ss{t}")
                    nc.vector.tensor_reduce(ss, sq, mybir.AxisListType.X, OP.add)
                    rn = _rsqrt(nc, sa, consts, ss[:], NH, f"ekrn{t}")
                    ek = sa.tile([P, NH, HD], F32R, tag="eout", bufs=2,
                                 name=f"ek{t}")
                    nc.vector.tensor_tensor(
                        ek, raw, rn[:, :, None].to_broadcast((P, NH, HD)),
                        OP.mult)
                    if debug and t == 0:
                        nc.sync.dma_start(
                            dbg["ek0"].ap(),
                            ek[:].rearrange("p h d -> p (h d)").bitcast(F32))
                        nc.sync.dma_start(
                            dbg["v0"].ap(),
                            v_sb[:].rearrange("p h d -> p (h d)").bitcast(F32))

                    # kv state accumulation (start only on first mm per bank)
                    for h in range(NH):
                        nc.tensor.matmul(
                            kv_ps[:, h * HD:(h + 1) * HD],
                            ek[:, h, :], v_sb[:, h, :],
                            start=(t == 0 and h % 8 == 0), stop=(t == NT - 1),
                            skip_group_check=True)

                # kv state -> SBUF (f32r), packed 2 heads per 128 partitions
                kvv = kv_ps[:].rearrange("p (a r d) -> p a r d", r=2, d=HD)
                kvb = kv_sb[:].rearrange("p (a d) -> p a d", d=HD)
                nc.vector.tensor_copy(kvb[0:HD], kvv[:, :, 0, :])
                nc.vector.tensor_copy(kvb[HD:P], kvv[:, :, 1, :])
                if debug:
                    kvstage = sa.tile([HD, NH * HD], F32, name="kvstage")
                    nc.vector.tensor_copy(kvstage, kv_ps)
                    nc.sync.dma_start(dbg["kv"].ap(), kvstage)

            # ------------- pass B: q -> ctx -> out ------------------------
            wq_sb = wpool.tile([P, NC, HID], F32R, tag="wA", name="wq_sb")
            nc.sync.dma_start(
                wq_sb, wqt_d.ap().rearrange("(c p) j -> p c j", p=P).bitcast(F32R))
            wo_sb = wpool.tile([P, NC, HID], F32R, tag="wB", name="wo_sb")
            nc.sync.dma_start(
                wo_sb, wot_d.ap().rearrange("(c p) j -> p c j", p=P).bitcast(F32R))

            with tc.tile_pool(name="sbufB", bufs=1) as sb, \
                 tc.tile_pool(name="psumB", bufs=1, space="PSUM") as pb:
                for ch in range(NCHUNKS):
                    eqT = sb.tile([P, NC, CHUNK * P], F32R, tag="eqT", bufs=1,
                                  name=f"eqT{ch}")
                    for tl in range(CHUNK):
                        t = ch * CHUNK + tl
                        zt_sb = sb.tile([P, HID], F32R, tag="zt", bufs=2,
                                        name=f"zt{t}")
                        nc.scalar.dma_start(zt_sb, zt_d.ap()[t].bitcast(F32R))

                        def zt_lhs(c, zt_sb=zt_sb):
                            return zt_sb[:, c * P:(c + 1) * P]

                        raw = sb.tile([P, NH, HD], F32, tag="eraw", bufs=2,
                                      name=f"eqraw{t}")
                        for half in range(2):
                            q_ps = pb.tile([P, 512], F32, tag="qh", bufs=2,
                                           name=f"q_ps{t}_{half}")
                            proj_half(q_ps, zt_lhs, wq_sb, half)
                            _elu_half(nc, sb, q_ps, brep["bq"], half, raw,
                                      f"eq{t}_{half}")

                        sq = sb.tile([P, NH, HD], F32, tag="esq", bufs=2,
                                     name=f"sqB{t}")
                        nc.scalar.activation(
                            sq[:].rearrange("p h d -> p (h d)"),
                            raw[:].rearrange("p h d -> p (h d)"), AF.Square)
                        ss = sb.tile([P, NH], F32, tag="ess", bufs=4,
                                     name=f"ssB{t}")
                        nc.vector.tensor_reduce(ss, sq, mybir.AxisListType.X,
                                                OP.add)
                        rn = _rsqrt(nc, sb, consts, ss[:], NH, f"eqrn{t}")
                        eq = sb.tile([P, NH, HD], F32R, tag="eout", bufs=2,
                                     name=f"eq{t}")
                        nc.vector.tensor_tensor(
                            eq, raw, rn[:, :, None].to_broadcast((P, NH, HD)),
                            OP.mult)
                        eqf = eq[:].rearrange("p h d -> p (h d)")
                        if debug and t == 0:
                            nc.sync.dma_start(dbg["eq0"].ap(), eqf.bitcast(F32))

                        for half in range(2):
                            tp = pb.tile([P, 4 * P], F32, tag="tp", bufs=2,
                                         name=f"tpB_{t}_{half}")
                            for b_ in range(4):
                                blk = half * 4 + b_
                                nc.tensor.transpose(
                                    tp[:, b_ * P:(b_ + 1) * P].bitcast(F32R),
                                    eqf[:, blk * P:(blk + 1) * P], ident_r)
                            nc.vector.tensor_copy(
                                eqT[:, half * 4:(half + 1) * 4,
                                    tl * P:(tl + 1) * P], tp)

                    # ctx^T for this chunk: per j-tile two row-tiled 64-K mms
                    ctxT = sb.tile([P, NC, CHUNK * P], F32R, tag="ctxT", bufs=1,
                                   name=f"ctxT{ch}")
                    for jt in range(NC):
                        c_pse = pb.tile([HD, CHUNK * P], F32, tag="ctxe",
                                        bufs=1, name=f"c_pse{ch}_{jt}")
                        c_pso = pb.tile([HD, CHUNK * P], F32, tag="ctxo",
                                        bufs=1, name=f"c_pso{ch}_{jt}")
                        nc.tensor.matmul(
                            c_pse, kv_sb[0:HD, jt * HD:(jt + 1) * HD],
                            eqT[0:HD, jt, :], start=True, stop=True)
                        nc.tensor.matmul(
                            c_pso, kv_sb[HD:P, jt * HD:(jt + 1) * HD],
                            eqT[HD:P, jt, :], start=True, stop=True)
                        nc.scalar.copy(ctxT[0:HD, jt, :], c_pse)
                        nc.scalar.copy(ctxT[HD:P, jt, :], c_pso)

                    for tl in range(CHUNK):
                        t = ch * CHUNK + tl

                        def ctx_lhs(c, ctxT=ctxT, tl=tl):
                            return ctxT[:, c, tl * P:(tl + 1) * P]

                        x_t2 = sb.tile([P, HID], F32, tag="x2", bufs=2,
                                       name=f"x2_{t}")
                        nc.sync.dma_start(x_t2, x_d.ap()[t * P:(t + 1) * P, :])
                        xb2 = sb.tile([P, HID], F32, tag="xb2", bufs=2,
                                      name=f"xb2_{t}")
                        nc.gpsimd.tensor_tensor(xb2, x_t2, brep["bo"], OP.add)
                        out_sb = sb.tile([P, HID], F32, tag="osb", bufs=2,
                                         name=f"out_{t}")
                        for half in range(2):
                            o_ps = pb.tile([P, 512], F32, tag="oh", bufs=2,
                                           name=f"o_ps{t}_{half}")
                            proj_half(o_ps, ctx_lhs, wo_sb, half)
                            sl = slice(half * 512, (half + 1) * 512)
                            nc.vector.tensor_tensor(
                                out_sb[:, sl], o_ps, xb2[:, sl], OP.add)
                        nc.gpsimd.dma_start(
                            out_d.ap()[t * P:(t + 1) * P, :], out_sb)

    nc.compile()
    return nc


_RUNNER = {}
_NC_CACHE = None


def _get_runner(loop_n=1):
    global _NC_CACHE
    if loop_n in _RUNNER:
        return _RUNNER[loop_n]

    import jax
    from jax.sharding import Mesh, PartitionSpec
    from jax.experimental.shard_map import shard_map
    from concourse.bass2jax import _bass_exec_p, install_neuronx_cc_hook

    install_neuronx_cc_hook()
    nc = build_nc(loop_n=loop_n)
    if loop_n == 1:
        _NC_CACHE = nc

    in_names = []
    out_names = []
    out_avals = []
    for alloc in nc.m.functions[0].allocations:
        if not isinstance(alloc, mybir.MemoryLocationSet):
            continue
        name = alloc.memorylocations[0].name
        if alloc.kind == "ExternalInput":
            in_names.append(name)
        elif alloc.kind == "ExternalOutput":
            out_names.append(name)
            out_avals.append(
                jax.core.ShapedArray(tuple(alloc.tensor_shape),
                                     mybir.dt.np(alloc.dtype)))
    n_params = len(in_names)
    all_in_names = in_names + out_names

    def _body(*args):
        outs = _bass_exec_p.bind(
            *args,
            out_avals=tuple(out_avals),
            in_names=tuple(all_in_names),
            out_names=tuple(out_names),
            lowering_input_output_aliases=(),
            sim_require_finite=True,
            sim_require_nnan=True,
            nc=nc,
        )
        return tuple(outs)

    devices = jax.devices()[:B]
    mesh = Mesh(np.asarray(devices), ("core",))
    n_outs = len(out_names)
    fn = jax.jit(
        shard_map(
            _body, mesh=mesh,
            in_specs=(PartitionSpec("core"),) * (n_params + n_outs),
            out_specs=(PartitionSpec("core"),) * n_outs,
            check_rep=False,
        ),
        keep_unused=True,
    )
    _RUNNER[loop_n] = (fn, in_names, out_names, out_avals)
    return _RUNNER[loop_n]


def prep_inputs(input_tensor, attention_mask, ln_gamma, ln_beta,
                Wq, bq, Wk, bk, Wv, bv, Wo, bo):
    """Host-side static prep: transpose weights, fold gamma/beta/scale."""
    f = np.float32
    x = np.ascontiguousarray(np.asarray(input_tensor, f))
    g = np.asarray(ln_gamma, f)
    be = np.asarray(ln_beta, f)
    Wq = np.asarray(Wq, f); Wk = np.asarray(Wk, f)
    Wv = np.asarray(Wv, f); Wo = np.asarray(Wo, f)
    wqt = np.ascontiguousarray((Wq * g[None, :]).T)        # [i, j]
    wkt = np.ascontiguousarray(Wk.T)
    wvt = np.ascontiguousarray(Wv.T)
    wot = np.ascontiguousarray(Wo.T * np.float32(1.0 / np.sqrt(HD)))
    bq_eff = (np.asarray(bq, f) + be @ Wq.T).astype(f)
    per_core = {
        "wqt": wqt, "wkt": wkt, "wvt": wvt, "wot": wot,
        "bq": bq_eff.reshape(1, HID),
        "bk": np.asarray(bk, f).reshape(1, HID),
        "bv": np.asarray(bv, f).reshape(1, HID),
        "bo": np.asarray(bo, f).reshape(1, HID),
    }
    return x, per_core


def kernel(**inputs) -> np.ndarray:
    x, per_core = prep_inputs(**inputs)
    fn, in_names, out_names, out_avals = _get_runner()

    concat_in = []
    for name in in_names:
        if name == "x":
            concat_in.append(x.reshape(B * S, HID))
        else:
            concat_in.append(np.concatenate([per_core[name]] * B, axis=0))
    concat_zeros = [
        np.zeros((B * av.shape[0], *av.shape[1:]), av.dtype) for av in out_avals
    ]
    out_arrs = fn(*concat_in, *concat_zeros)
    out = np.asarray(out_arrs[out_names.index("out")])
    return out.reshape(B, S, HID)



# revision 9
# speedup vs baseline: 1.2161x; 1.2161x over previous
"""Trainium2 Bass kernel for nn_MultiHeadAttention_84791244358011.

Linear (ELU feature-map) attention:
    x_norm = LayerNorm(x)                      # eps=1e-12
    q = x_norm @ Wq.T + bq ; k,v = x @ W.T + b # per-head [S, 64]
    eq/ek = l2norm(elu(q/k)) per token over head_dim
    kv = ek^T @ v per head [64, 64]; ctx = eq @ kv / 8
    out = ctx @ Wo.T + bo + x

Sharding: data-parallel over batch B=8 — one batch element per NeuronCore,
no collectives. All matmul operands are bf16 (PSUM accumulation stays f32).

Single pass over the 32 token tiles (vs. the old two-pass + DRAM spill):
  - x^T comes straight from DRAM through the DMA xbar transpose (one
    InstDmaTransposeAnt per tile) — no PE transposes, no PSUM copies.
  - LayerNorm is folded into the Q projection:
        q = rstd * (x @ wqt - mu x rw) + bq_eff,   rw_j = sum_i wqt[i,j]
    mu enters as a rank-1 matmul (mu row x -rw row); rstd enters as the
    per-partition *scale* operand of the ELU's two Relu activations.
    The mu row itself is a ones-column matmul against x^T, borrowed into
    the q0 PSUM bank before the q accumulation starts.
  - elu(x)+1 == relu(x) + exp(min(x,0)) exactly, so the ELU is
    relu(-x) -> exp(-.) on ACT, relu(x) on ACT, one add on Pool; the
    "-1" folds into Square's bias (sum-of-squares) and a tensor_scalar.
  - all rsqrt via exp(-0.5*ln(.)) on ACT: ln and exp live in the same
    activation table set (natural_log_exp_and_others) so there are no
    1.28us table swaps, and no DVE Newton chains.
  - the k-side l2 norm is applied to V instead of K (kv = sum ek x v is
    bilinear), saving a broadcast multiply from PSUM.
  - per-head kv state accumulates into a single PSUM bank: even heads at
    partitions 0-63, odd heads at 64-127 via tile_position=(0,64).
  - eq is written bf16 and xbar-transposed SBUF->SBUF into a persistent
    [128, 8, 4096] eqT buffer; pass 2 (ctx = kv @ eqT, out = ctx^T @ wot
    + x) runs from SBUF with no transposes.
  - kv accumulation matmuls for tile t issue after tile t+1's projection
    matmuls so the PE never waits on the k-norm chain.

Biases are all zero for this problem's inputs; a with_bias variant adds
rank-1 (ones x bias) matmuls into each projection's PSUM group.
"""

import contextlib

import numpy as np

import concourse.bass as bass
import concourse.mybir as mybir
import concourse.tile as tile
from concourse import bacc

B, S, HID = 8, 4096, 1024
NH, HD = 16, 64
P = 128
NT = S // P            # 32 token tiles
NC = HID // P          # 8 feature chunks
CHUNK = 4              # token tiles per ctx chunk (512 tokens)
NCHUNKS = NT // CHUNK
LN_EPS = 1e-12

F32 = mybir.dt.float32
BF16 = mybir.dt.bfloat16
AF = mybir.ActivationFunctionType
OP = mybir.AluOpType
AX = mybir.AxisListType


def build_nc(with_bias=False, loop_n=1):
    nc = bacc.Bacc("TRN2", target_bir_lowering=False, enable_partition_id=False)

    x_d = nc.dram_tensor("x", [S, HID], BF16, kind="ExternalInput")
    wqt_d = nc.dram_tensor("wqt", [HID, HID], BF16, kind="ExternalInput")
    wkt_d = nc.dram_tensor("wkt", [HID, HID], BF16, kind="ExternalInput")
    wvt_d = nc.dram_tensor("wvt", [HID, HID], BF16, kind="ExternalInput")
    wot_d = nc.dram_tensor("wot", [HID, HID], BF16, kind="ExternalInput")
    nrw_d = nc.dram_tensor("nrw", [1, HID], BF16, kind="ExternalInput")
    bias_d = {}
    if with_bias:
        for nm in ("bq", "bk", "bv", "bo"):
            bias_d[nm] = nc.dram_tensor(nm, [1, HID], BF16, kind="ExternalInput")
    out_d = nc.dram_tensor("out", [S, HID], F32, kind="ExternalOutput")

    with tile.TileContext(nc) as tc, contextlib.ExitStack() as ctx:
        persist = ctx.enter_context(tc.tile_pool(name="persist", bufs=1))
        wpool = ctx.enter_context(tc.tile_pool(name="weights", bufs=1))

        ones_col = persist.tile([P, 1], BF16)
        nc.gpsimd.memset(ones_col, 1.0)
        eps_c = persist.tile([P, 1], F32)
        nc.gpsimd.memset(eps_c, LN_EPS)
        negone_c = persist.tile([P, 1], F32)
        nc.gpsimd.memset(negone_c, -1.0)
        kv_sb = persist.tile([P, NC * HD], BF16)      # packed kv state
        eqT = persist.tile([P, NC, S], BF16)          # transposed eq, full S
        nrw_sb = persist.tile([1, HID], BF16)
        nc.sync.dma_start(nrw_sb, nrw_d.ap())
        brow = {}
        if with_bias:
            ones_row = persist.tile([1, P], BF16)
            nc.gpsimd.memset(ones_row, 1.0)
            for nm, d in bias_d.items():
                t_ = persist.tile([1, HID], BF16, name=f"{nm}_row")
                nc.sync.dma_start(t_, d.ap())
                brow[nm] = t_

        _loop = tc.For_i(0, loop_n, 1) if loop_n > 1 else contextlib.nullcontext(0)
        with _loop:
            # ------------- pass 1: k/v/q, kv state, eqT -------------
            wk_sb = wpool.tile([P, NC, HID], BF16, tag="wA", name="wk_sb")
            nc.sync.dma_start(wk_sb, wkt_d.ap().rearrange("(c p) j -> p c j", p=P))
            wv_sb = wpool.tile([P, NC, HID], BF16, tag="wB", name="wv_sb")
            nc.sync.dma_start(wv_sb, wvt_d.ap().rearrange("(c p) j -> p c j", p=P))
            wq_sb = wpool.tile([P, NC, HID], BF16, tag="wC", name="wq_sb")
            nc.sync.dma_start(wq_sb, wqt_d.ap().rearrange("(c p) j -> p c j", p=P))

            with tc.tile_pool(name="sbufA", bufs=1) as sa, \
                 tc.tile_pool(name="psumA", bufs=1, space="PSUM") as pa:
                kv_ps = pa.tile([P, NC * HD], F32, tag="kv", name="kv_ps")
                pend_kv = []           # deferred kv matmuls (ek, vs) per tile

                def flush_kv(last):
                    if not pend_kv:
                        return
                    first, ek, vs = pend_kv.pop()
                    for a in range(NC):
                        nc.tensor.matmul(
                            kv_ps[0:HD, a * HD:(a + 1) * HD],
                            ek[:, 2 * a, :], vs[:, 2 * a, :],
                            start=(first and a == 0), stop=last,
                            tile_position=(0, 0), skip_group_check=True)
                        nc.tensor.matmul(
                            kv_ps[HD:P, a * HD:(a + 1) * HD],
                            ek[:, 2 * a + 1, :], vs[:, 2 * a + 1, :],
                            start=False, stop=last,
                            tile_position=(0, 64), skip_group_check=True)

                for t in range(NT):
                    tok = slice(t * P, (t + 1) * P)
                    x_t = sa.tile([P, HID], BF16, tag="x", bufs=3, name=f"x_{t}")
                    nc.scalar.dma_start(x_t, x_d.ap()[tok, :])
                    xT = sa.tile([P, NC, P], BF16, tag="xT", bufs=3,
                                 name=f"xT_{t}")
                    nc.sync.dma_start_transpose(xT, x_d.ap()[tok, :])

                    # LayerNorm stats; rstd = exp(-0.5*ln(var+eps))
                    stats = sa.tile([P, 2, 6], F32, tag="st", bufs=3,
                                    name=f"st_{t}")
                    xg = x_t[:].rearrange("p (g d) -> p g d", g=2)
                    for g in range(2):
                        nc.vector.bn_stats(stats[:, g, :], xg[:, g, :])
                    mv = sa.tile([P, 2], F32, tag="mv", bufs=3, name=f"mv_{t}")
                    nc.vector.bn_aggr(mv, stats)
                    lnv = sa.tile([P, 1], F32, tag="lnv", bufs=3, name=f"lnv_{t}")
                    nc.scalar.activation(lnv, mv[:, 1:2], AF.Ln, bias=eps_c[:, 0:1])
                    rstd = sa.tile([P, 1], F32, tag="rsd", bufs=3, name=f"rsd_{t}")
                    nc.scalar.activation(rstd, lnv, AF.Exp, scale=-0.5)
                    nrstd = sa.tile([P, 1], F32, tag="nrs", bufs=3, name=f"nrs_{t}")
                    nc.vector.tensor_scalar(nrstd, rstd, -1.0, None, OP.mult)

                    # mu row via ones-matmul, borrowed into the q0 psum bank
                    q_ps = [pa.tile([P, 512], F32, tag="qh", bufs=2,
                                    name=f"q_ps{t}_{h}") for h in range(2)]
                    for c in range(NC):
                        nc.tensor.matmul(q_ps[0][0:1, 0:P], ones_col,
                                         xT[:, c, :],
                                         start=(c == 0), stop=(c == NC - 1),
                                         skip_group_check=True)
                    mu_row = sa.tile([1, P], BF16, tag="mu", bufs=3,
                                     name=f"mu_{t}")
                    nc.vector.tensor_scalar(mu_row, q_ps[0][0:1, 0:P],
                                            1.0 / HID, None, OP.mult)

                    def proj(ps, w_sb, half, extras=()):
                        sl = slice(half * 512, (half + 1) * 512)
                        for c in range(NC):
                            nc.tensor.matmul(
                                ps, xT[:, c, :], w_sb[:, c, sl],
                                start=(c == 0),
                                stop=(c == NC - 1 and not extras),
                                skip_group_check=True)
                        for i, (lhs, rhs_row) in enumerate(extras):
                            nc.tensor.matmul(
                                ps, lhs, rhs_row[0:1, sl],
                                start=False, stop=(i == len(extras) - 1),
                                skip_group_check=True)

                    # ---- K ----
                    k_ps = [pa.tile([P, 512], F32, tag="kh", bufs=2,
                                    name=f"k_ps{t}_{h}") for h in range(2)]
                    kex = [(ones_row, brow["bk"])] if with_bias else []
                    w1k = sa.tile([P, HID], BF16, tag="w1k", bufs=3,
                                  name=f"w1k_{t}")
                    for half in range(2):
                        proj(k_ps[half], wk_sb, half, kex)
                        hs = slice(half * 512, (half + 1) * 512)
                        r = sa.tile([P, 512], BF16, tag="kr", bufs=3,
                                    name=f"kr_{t}_{half}")
                        nc.scalar.activation(r, k_ps[half], AF.Relu, scale=-1.0)
                        e = sa.tile([P, 512], BF16, tag="ke", bufs=3,
                                    name=f"ke_{t}_{half}")
                        nc.scalar.activation(e, r, AF.Exp, scale=-1.0)
                        m = sa.tile([P, 512], BF16, tag="km", bufs=3,
                                    name=f"km_{t}_{half}")
                        nc.scalar.activation(m, k_ps[half], AF.Relu)
                        nc.gpsimd.tensor_tensor(w1k[:, hs], m, e, OP.add)
                    sqk = sa.tile([P, NH, HD], BF16, tag="sqk", bufs=2,
                                  name=f"sqk_{t}")
                    nc.scalar.activation(
                        sqk[:].rearrange("p h d -> p (h d)"), w1k, AF.Square,
                        bias=negone_c[:, 0:1])
                    ssk = sa.tile([P, NH], F32, tag="ssk", bufs=3,
                                  name=f"ssk_{t}")
                    nc.vector.tensor_reduce(ssk, sqk, AX.X, OP.add)
                    lnk = sa.tile([P, NH], F32, tag="lnk", bufs=3,
                                  name=f"lnk_{t}")
                    nc.scalar.activation(lnk, ssk, AF.Ln)
                    rnk = sa.tile([P, NH], F32, tag="rnk", bufs=3,
                                  name=f"rnk_{t}")
                    nc.scalar.activation(rnk, lnk, AF.Exp, scale=-0.5)
                    ek = sa.tile([P, NH, HD], BF16, tag="ek", bufs=3,
                                 name=f"ek_{t}")
                    nc.vector.tensor_scalar(
                        ek[:].rearrange("p h d -> p (h d)"), w1k, 1.0, None,
                        OP.subtract)

                    # ---- V (k-norm folded in) ----
                    v_ps = [pa.tile([P, 512], F32, tag="vh", bufs=3,
                                    name=f"v_ps{t}_{h}") for h in range(2)]
                    vex = [(ones_row, brow["bv"])] if with_bias else []
                    vs = sa.tile([P, NH, HD], BF16, tag="vs", bufs=3,
                                 name=f"vs_{t}")
                    for half in range(2):
                        proj(v_ps[half], wv_sb, half, vex)
                        hh = slice(half * 8, (half + 1) * 8)
                        nc.vector.tensor_tensor(
                            vs[:, hh, :],
                            v_ps[half][:].rearrange("p (h d) -> p h d", d=HD),
                            rnk[:, hh, None].to_broadcast((P, 8, HD)),
                            OP.mult)

                    # ---- Q (LayerNorm folded in) ----
                    qex = [(mu_row, nrw_sb)]
                    if with_bias:
                        qex.append((ones_row, brow["bq"]))
                    w1q = sa.tile([P, HID], BF16, tag="w1q", bufs=3,
                                  name=f"w1q_{t}")
                    for half in range(2):
                        proj(q_ps[half], wq_sb, half, qex)
                        hs = slice(half * 512, (half + 1) * 512)
                        r = sa.tile([P, 512], BF16, tag="qr", bufs=3,
                                    name=f"qr_{t}_{half}")
                        nc.scalar.activation(r, q_ps[half], AF.Relu,
                                             scale=nrstd[:, 0:1])
                        e = sa.tile([P, 512], BF16, tag="qe", bufs=3,
                                    name=f"qe_{t}_{half}")
                        nc.scalar.activation(e, r, AF.Exp, scale=-1.0)
                        m = sa.tile([P, 512], BF16, tag="qm", bufs=3,
                                    name=f"qm_{t}_{half}")
                        nc.scalar.activation(m, q_ps[half], AF.Relu,
                                             scale=rstd[:, 0:1])
                        nc.gpsimd.tensor_tensor(w1q[:, hs], m, e, OP.add)
                    sqq = sa.tile([P, NH, HD], BF16, tag="sqq", bufs=2,
                                  name=f"sqq_{t}")
                    nc.scalar.activation(
                        sqq[:].rearrange("p h d -> p (h d)"), w1q, AF.Square,
                        bias=negone_c[:, 0:1])
                    ssq = sa.tile([P, NH], F32, tag="ssq", bufs=3,
                                  name=f"ssq_{t}")
                    nc.vector.tensor_reduce(ssq, sqq, AX.X, OP.add)
                    lnq = sa.tile([P, NH], F32, tag="lnq", bufs=3,
                                  name=f"lnq_{t}")
                    nc.scalar.activation(lnq, ssq, AF.Ln)
                    rnq = sa.tile([P, NH], BF16, tag="rnq", bufs=3,
                                  name=f"rnq_{t}")
                    nc.scalar.activation(rnq, lnq, AF.Exp, scale=-0.5)
                    eqp = sa.tile([P, NH, HD], BF16, tag="eqp", bufs=2,
                                  name=f"eqp_{t}")
                    nc.vector.tensor_scalar(
                        eqp[:].rearrange("p h d -> p (h d)"), w1q, 1.0, None,
                        OP.subtract)
                    eq = sa.tile([P, NH, HD], BF16, tag="eq", bufs=2,
                                 name=f"eq_{t}")
                    nc.vector.tensor_tensor(
                        eq, eqp, rnq[:, :, None].to_broadcast((P, NH, HD)),
                        OP.mult)
                    nc.sync.dma_start_transpose(
                        eqT[:, :, tok], eq[:].rearrange("p h d -> p (h d)"))

                    # kv accumulation for the previous tile (PE never waits)
                    flush_kv(last=False)
                    pend_kv.append((t == 0, ek, vs))
                flush_kv(last=True)
                nc.vector.tensor_copy(kv_sb, kv_ps)

            # ------------- pass 2: ctx -> out -----------------------
            wo_sb = wpool.tile([P, NC, HID], BF16, tag="wD", name="wo_sb")
            nc.sync.dma_start(wo_sb, wot_d.ap().rearrange("(c p) j -> p c j", p=P))

            with tc.tile_pool(name="sbufB", bufs=1) as sb, \
                 tc.tile_pool(name="psumB", bufs=1, space="PSUM") as pb:
                for ch in range(NCHUNKS):
                    win = slice(ch * CHUNK * P, (ch + 1) * CHUNK * P)
                    ctxT = sb.tile([P, NC, CHUNK * P], BF16, tag="ctxT",
                                   bufs=2, name=f"ctxT{ch}")
                    for jt in range(NC):
                        c_pse = pb.tile([HD, CHUNK * P], F32, tag="ctxe",
                                        bufs=2, name=f"c_pse{ch}_{jt}")
                        c_pso = pb.tile([HD, CHUNK * P], F32, tag="ctxo",
                                        bufs=2, name=f"c_pso{ch}_{jt}")
                        nc.tensor.matmul(
                            c_pse, kv_sb[0:HD, jt * HD:(jt + 1) * HD],
                            eqT[0:HD, jt, win], start=True, stop=True)
                        nc.tensor.matmul(
                            c_pso, kv_sb[HD:P, jt * HD:(jt + 1) * HD],
                            eqT[HD:P, jt, win], start=True, stop=True)
                        nc.scalar.copy(ctxT[0:HD, jt, :], c_pse)
                        nc.scalar.copy(ctxT[HD:P, jt, :], c_pso)

                    for tl in range(CHUNK):
                        t = ch * CHUNK + tl
                        tok = slice(t * P, (t + 1) * P)
                        x2 = sb.tile([P, HID], BF16, tag="x2", bufs=2,
                                     name=f"x2_{t}")
                        nc.scalar.dma_start(x2, x_d.ap()[tok, :])
                        out_sb = sb.tile([P, HID], F32, tag="osb", bufs=2,
                                         name=f"out_{t}")
                        for half in range(2):
                            hs = slice(half * 512, (half + 1) * 512)
                            o_ps = pb.tile([P, 512], F32, tag="oh", bufs=2,
                                           name=f"o_ps{t}_{half}")
                            for c in range(NC):
                                nc.tensor.matmul(
                                    o_ps, ctxT[:, c, tl * P:(tl + 1) * P],
                                    wo_sb[:, c, hs],
                                    start=(c == 0),
                                    stop=(c == NC - 1 and not with_bias),
                                    skip_group_check=True)
                            if with_bias:
                                nc.tensor.matmul(
                                    o_ps, ones_row, brow["bo"][0:1, hs],
                                    start=False, stop=True,
                                    skip_group_check=True)
                            nc.vector.tensor_tensor(
                                out_sb[:, hs], o_ps, x2[:, hs], OP.add)
                        nc.gpsimd.dma_start(out_d.ap()[tok, :], out_sb)

    nc.compile()
    _force_single_act_table(nc)
    return nc


def _force_single_act_table(nc):
    """Retarget all activation-table loads to natural_log_exp_and_others
    (which contains every function this kernel uses: ln, exp, relu, square,
    copy) and drop the now-redundant reloads. The insertion pass picks the
    first table containing each function, which thrashes exp<->ln at 1.28us
    per reload, ~6x per token tile."""
    from concourse.hw_specs import get_activation_tables

    names = list(get_activation_tables(nc.m.arch).keys())
    target = names.index("natural_log_exp_and_others")
    kept_one = False
    for b in nc.main_func.blocks:
        keep = []
        for i in b.instructions:
            if isinstance(i, mybir.InstLoadActFuncSet):
                si = getattr(i, "sync_info", None)
                has_sync = si is not None and (
                    len(si.on_wait) > 0 or len(si.on_update) > 0)
                i.act_func_set_id = target
                if not kept_one or has_sync:
                    kept_one = True
                    keep.append(i)
            else:
                keep.append(i)
        b.instructions[:] = keep


_RUNNER = {}


def _get_runner(loop_n=1, with_bias=False):
    key = (loop_n, with_bias)
    if key in _RUNNER:
        return _RUNNER[key]

    import jax
    from jax.sharding import Mesh, PartitionSpec
    from jax.experimental.shard_map import shard_map
    from concourse.bass2jax import _bass_exec_p, install_neuronx_cc_hook

    install_neuronx_cc_hook()
    nc = build_nc(with_bias=with_bias, loop_n=loop_n)

    in_names = []
    out_names = []
    out_avals = []
    for alloc in nc.m.functions[0].allocations:
        if not isinstance(alloc, mybir.MemoryLocationSet):
            continue
        name = alloc.memorylocations[0].name
        if alloc.kind == "ExternalInput":
            in_names.append(name)
        elif alloc.kind == "ExternalOutput":
            out_names.append(name)
            out_avals.append(
                jax.core.ShapedArray(tuple(alloc.tensor_shape),
                                     mybir.dt.np(alloc.dtype)))
    n_params = len(in_names)
    all_in_names = in_names + out_names

    def _body(*args):
        outs = _bass_exec_p.bind(
            *args,
            out_avals=tuple(out_avals),
            in_names=tuple(all_in_names),
            out_names=tuple(out_names),
            lowering_input_output_aliases=(),
            sim_require_finite=True,
            sim_require_nnan=True,
            nc=nc,
        )
        return tuple(outs)

    devices = jax.devices()[:B]
    mesh = Mesh(np.asarray(devices), ("core",))
    n_outs = len(out_names)
    fn = jax.jit(
        shard_map(
            _body, mesh=mesh,
            in_specs=(PartitionSpec("core"),) * (n_params + n_outs),
            out_specs=(PartitionSpec("core"),) * n_outs,
            check_rep=False,
        ),
        keep_unused=True,
    )
    _RUNNER[key] = (fn, in_names, out_names, out_avals)
    return _RUNNER[key]


def prep_inputs(input_tensor, attention_mask, ln_gamma, ln_beta,
                Wq, bq, Wk, bk, Wv, bv, Wo, bo):
    """Host-side static prep: transpose weights, fold gamma/beta/scale."""
    import ml_dtypes
    bf = ml_dtypes.bfloat16
    f = np.float32
    x = np.asarray(input_tensor, f)
    g = np.asarray(ln_gamma, f)
    be = np.asarray(ln_beta, f)
    Wq = np.asarray(Wq, f); Wk = np.asarray(Wk, f)
    Wv = np.asarray(Wv, f); Wo = np.asarray(Wo, f)
    wqt = np.ascontiguousarray((Wq * g[None, :]).T)        # [i, j]
    nrw = -wqt.sum(axis=0, keepdims=True)                  # [1, j]
    bq_eff = (np.asarray(bq, f) + be @ Wq.T).astype(f)
    per_core = {
        "wqt": wqt.astype(bf),
        "wkt": np.ascontiguousarray(Wk.T).astype(bf),
        "wvt": np.ascontiguousarray(Wv.T).astype(bf),
        "wot": np.ascontiguousarray(Wo.T * np.float32(1.0 / np.sqrt(HD))).astype(bf),
        "nrw": nrw.astype(bf),
    }
    biases = {"bq": bq_eff, "bk": np.asarray(bk, f),
              "bv": np.asarray(bv, f), "bo": np.asarray(bo, f)}
    has_bias = any(np.any(v) for v in biases.values())
    if has_bias:
        for nm, v in biases.items():
            per_core[nm] = v.reshape(1, HID).astype(bf)
    return np.ascontiguousarray(x.astype(bf)), per_core, has_bias


def kernel(**inputs) -> np.ndarray:
    x, per_core, has_bias = prep_inputs(**inputs)
    fn, in_names, out_names, out_avals = _get_runner(1, has_bias)

    concat_in = []
    for name in in_names:
        if name == "x":
            concat_in.append(x.reshape(B * S, HID))
        else:
            concat_in.append(np.concatenate([per_core[name]] * B, axis=0))
    concat_zeros = [
        np.zeros((B * av.shape[0], *av.shape[1:]), av.dtype) for av in out_avals
    ]
    out_arrs = fn(*concat_in, *concat_zeros)
    out = np.asarray(out_arrs[out_names.index("out")])
    return out.reshape(B, S, HID)
